# revision 1
# baseline (speedup 1.0000x reference)
"""Trainium2 Bass kernel for nn_ExchangeBlock (GNN message passing / e3nn-style
tensor-product edge block), SPMD across 8 NeuronCores.

Sharding: edges across the 8 cores; node features and params replicated.

v2 design notes:
- All row gathers use the 16-lane GPSIMD dma_gather ucode (512-1024 indices
  per instruction) instead of per-row indirect DMA descriptor generation.
  dma_gather takes int16 indices, so node tables are stored as paired rows
  (25000 x 2*rowlen), indexed by node_id>>1 with an on-chip parity select.
- Two activation-table phases: phase A (exp table) computes geometry + RBF
  for all blocks (sqrt via Newton rsqrt on the VectorEngine); phase B (silu
  table) does everything else; Sin (envelope cosine) lives in the silu set.
- The tensor product runs as outer-product features P[e,1344] built on DVE
  with broadcast access patterns (bf16 for the big 0e x 0e block), PE
  transposes of P chunks (bf16: single-pass, fp32 would split into 2 ops),
  and accumulated 128x128 matmuls against pre-scaled flattened weights.
- LayerNorm affine is folded into a widened dfilter matmul; biases are added
  on DVE straight into PSUM (avoids rank-1 bias matmuls on the PE).
"""

import sys

sys.path.insert(0, "/opt/trn_rl_repo")

import numpy as np
import ml_dtypes

import concourse.bass as bass
import concourse.mybir as mybir
import concourse.tile as tile
from concourse import bacc
from concourse.bass_utils import run_bass_kernel_spmd
from concourse.masks import make_identity

F32 = mybir.dt.float32
BF16 = mybir.dt.bfloat16
I32 = mybir.dt.int32
I16 = mybir.dt.int16
AF = mybir.ActivationFunctionType
OP = mybir.AluOpType

# Problem constants
L0, L1, L2 = 32, 16, 8
NS = 128
NB = 64
CUTOFF = 7.0
N_NODES = 50000
N_EDGES = 400000
NODE_DIM = 120
NCORES = 8

BLK = 512             # edges per block
SUB = 4               # 128-edge sub-tiles per block
P = 128
KTP = 1344            # 1024 + 256 + 64 contraction size
KPAD = 1408           # padded to 11 chunks of 128
NCHUNK = 11
RSQRT_MAGIC = 0x5F3759DF
NPAIR = N_NODES // 2  # 25000
XROW = 128            # padded node row (124 used)
PROW = 32             # padded pos row (4 used)

E_CORE = N_EDGES // NCORES                      # 50000
NBLOCKS = (E_CORE + BLK - 1) // BLK             # 98
E_PAD = NBLOCKS * BLK                           # 50176

_compiled = None


def _patch_walrus_dge_levels():
    """This walrus build compiles with DynamicDMA disabled by default, which
    makes dynamic-offset DMAs crash the exec unit. Append the full
    --dge-levels set to every walrus invocation."""
    import concourse.bass_utils as _bu

    if getattr(_bu, "_dge_patched", False):
        return
    orig = _bu.run_command

    def patched(argv, **kw):
        if argv and "walrus_driver" in str(argv[0]) and not any(
            "dge-levels" in str(a) for a in argv
        ):
            argv = list(argv) + [
                "--dge-levels=io,spill_reload,scalar_dynamic_offset,"
                "vector_dynamic_offsets,dynamic_size,dst_reduce,transpose"
            ]
        return orig(argv, **kw)

    _bu.run_command = patched
    _bu._dge_patched = True


_patch_walrus_dge_levels()


def _patch_drain_and_barrier():
    """The final Tile drain runs on the SP engine, whose Drain lowering in this
    walrus build has no free sync-wait slots (its HWDGE queue waits fill them).
    Hoist the tile-clock waits onto dedicated nop instructions emitted just
    before the drain, one wait per nop."""
    if getattr(tile.TileContext, "_dab_patched", False):
        return

    def patched(self, tick_clock, wait_clock):
        nc = self.nc
        nops = [nc.sync.nop() for _ in range(32)]
        drain_inst = nc.sync.drain()
        from concourse.tile import ScopedClock

        wait_clock.add_sem_waits(
            drain_inst.ins, ScopedClock({None: tick_clock.global_clock})
        )
        si = drain_inst.ins.sync_info
        waits = list(si.on_wait) if si and si.on_wait else []
        if waits:
            assert len(waits) <= len(nops), f"{len(waits)} waits > nop slots"
            si.on_wait = []
            for w, n in zip(waits, nops):
                n.ins.sync_info = mybir.SyncInfo(on_wait=[w], on_update=[])

        nc.all_engine_barrier()
        assert self.sems is not None
        popped = nc._tile_sem_poison_stack.pop()
        assert popped is self._sem_poison
        nc.clear_and_free_semaphores(list(self.sems.allocated().values()))
        nc.all_engine_barrier()

    tile.TileContext._drain_and_barrier = patched
    tile.TileContext._dab_patched = True


_patch_drain_and_barrier()


def _newton_rsqrt(nc, pool, u, n, magic_t, tag):
    """rsqrt(u) for u[:, :n] > 0 on the VectorEngine (no ScalarE table)."""
    bits = pool.tile([P, n], I32, tag=f"{tag}_b")
    nc.vector.tensor_copy(out=bits[:].bitcast(F32), in_=u)  # raw bit copy
    nc.vector.tensor_scalar(
        out=bits[:], in0=bits[:], scalar1=1, scalar2=None,
        op0=OP.arith_shift_right,
    )
    yb = pool.tile([P, n], I32, tag=f"{tag}_y")
    nc.vector.tensor_tensor(
        out=yb[:], in0=magic_t[:, 0:1].to_broadcast([P, n]), in1=bits[:],
        op=OP.subtract,
    )
    y = yb[:].bitcast(F32)
    t1 = pool.tile([P, n], F32, tag=f"{tag}_t1")
    for _ in range(3):
        nc.vector.tensor_mul(t1[:], y, y)
        nc.vector.tensor_mul(t1[:], t1[:], u)
        nc.vector.tensor_scalar(
            out=t1[:], in0=t1[:], scalar1=-0.5, scalar2=1.5, op0=OP.mult, op1=OP.add,
        )
        nc.vector.tensor_mul(y, y, t1[:])
    return yb


def _build(nblocks: int):
    import os
    stage = os.environ.get("K_STAGE", "full")
    nc = bacc.Bacc("TRN2", target_bir_lowering=False, debug=False)

    nodes_pair = nc.dram_tensor("nodes_pair", (NPAIR, 2 * XROW), F32, kind="ExternalInput").ap()
    pos_pair = nc.dram_tensor("pos_pair", (NPAIR, 2 * PROW), F32, kind="ExternalInput").ap()
    cell64 = nc.dram_tensor("cell64", (32, 64), F32, kind="ExternalInput").ap()
    xw16 = nc.dram_tensor("xw16", (nblocks, P, 64), I16, kind="ExternalInput").ap()
    gbw16 = nc.dram_tensor("gbw16", (nblocks, P, 32), I16, kind="ExternalInput").ap()
    par = nc.dram_tensor("par", (nblocks * BLK, 2), F32, kind="ExternalInput").ap()
    eshift = nc.dram_tensor("eshift", (nblocks * BLK, 3), F32, kind="ExternalInput").ap()
    wflat = nc.dram_tensor("wflat", (KPAD, NS), BF16, kind="ExternalInput").ap()
    dfw1 = nc.dram_tensor("dfw1", (NB, 128), BF16, kind="ExternalInput").ap()
    dfb1 = nc.dram_tensor("dfb1", (1, 128), F32, kind="ExternalInput").ap()
    dfw2gb = nc.dram_tensor("dfw2gb", (128, 256), BF16, kind="ExternalInput").ap()
    dfb2gb = nc.dram_tensor("dfb2gb", (1, 256), F32, kind="ExternalInput").ap()
    mlpw1 = nc.dram_tensor("mlpw1", (128, 512), BF16, kind="ExternalInput").ap()
    mlpb1 = nc.dram_tensor("mlpb1", (1, 512), F32, kind="ExternalInput").ap()
    w2row = nc.dram_tensor("w2row", (1, 512), BF16, kind="ExternalInput").ap()
    b2sc = nc.dram_tensor("b2sc", (1, 1), F32, kind="ExternalInput").ap()
    offs = nc.dram_tensor("offs", (1, NB), F32, kind="ExternalInput").ap()
    out = nc.dram_tensor("out", (nblocks * BLK,), F32, kind="ExternalOutput").ap()

    width = CUTOFF / (NB - 1)
    coeff = 0.5 / (width * width)
    sqc = float(np.sqrt(coeff))

    with tile.TileContext(nc) as tc:
        with (
            tc.tile_pool(name="const", bufs=1) as constp,
            tc.tile_pool(name="io", bufs=3) as iop,
            tc.tile_pool(name="geo", bufs=3) as geop,
            tc.tile_pool(name="pfeat", bufs=2) as pfp,
            tc.tile_pool(name="trsb", bufs=3) as trsbp,
            tc.tile_pool(name="work", bufs=3) as workp,
            tc.tile_pool(name="gbig", bufs=2) as gbigp,
            tc.tile_pool(name="acc", bufs=2) as accp,
            tc.tile_pool(name="ps_tr", bufs=2, space="PSUM") as ps_tr,
            tc.tile_pool(name="ps_mm", bufs=2, space="PSUM") as ps_mm,
            tc.tile_pool(name="ps_h", bufs=1, space="PSUM") as ps_h,
            tc.tile_pool(name="ps_df", bufs=1, space="PSUM") as ps_df,
            tc.tile_pool(name="ps_g", bufs=2, space="PSUM") as ps_g,
        ):
            # ---- resident constants ----
            identb = constp.tile([P, P], BF16)
            make_identity(nc, identb[:])
            eps_t = constp.tile([P, 1], F32)
            nc.vector.memset(eps_t[:], 1e-5)
            nhalfpi_t = constp.tile([P, 1], F32)
            nc.vector.memset(nhalfpi_t[:], float(-np.pi / 2))
            magic_t = constp.tile([P, 1], I32)
            nc.vector.memset(magic_t[:], RSQRT_MAGIC)

            w_sb = constp.tile([P, NCHUNK, P], BF16)
            nc.sync.dma_start(out=w_sb[:], in_=wflat.rearrange("(c p) w -> p c w", p=P))
            dfw1_sb = constp.tile([NB, 128], BF16)
            nc.sync.dma_start(out=dfw1_sb[:], in_=dfw1)
            dfw2gb_sb = constp.tile([128, 256], BF16)
            nc.sync.dma_start(out=dfw2gb_sb[:], in_=dfw2gb)
            mlpw1_sb = constp.tile([128, 512], BF16)
            nc.sync.dma_start(out=mlpw1_sb[:], in_=mlpw1)
            dfb1_rep = constp.tile([P, 128], F32)
            nc.sync.dma_start(out=dfb1_rep[:], in_=dfb1.to_broadcast([P, 128]))
            dfb2gb_rep = constp.tile([P, 256], F32)
            nc.sync.dma_start(out=dfb2gb_rep[:], in_=dfb2gb.to_broadcast([P, 256]))
            mlpb1_rep = constp.tile([P, 512], F32)
            nc.sync.dma_start(out=mlpb1_rep[:], in_=mlpb1.to_broadcast([P, 512]))
            w2rep_sb = constp.tile([P, 512], BF16)
            nc.sync.dma_start(out=w2rep_sb[:], in_=w2row.to_broadcast([P, 512]))
            b2_sb = constp.tile([P, 1], F32)
            nc.sync.dma_start(out=b2_sb[:], in_=b2sc.to_broadcast([P, 1]))
            offs_sb = constp.tile([P, NB], F32)
            nc.sync.dma_start(out=offs_sb[:], in_=offs.to_broadcast([P, NB]))

            # phase A -> phase B hand-off (resident)
            rbf_store = constp.tile([P, nblocks, SUB, NB], BF16)
            dist_store = constp.tile([P, nblocks, SUB], F32)

            # =========== Phase A: geometry + RBF (exp table) ===========
            for b in range(nblocks):
                e0 = b * BLK
                sl = slice(e0, e0 + BLK)
                xw = iop.tile([P, 64], I16, tag="xw")
                nc.sync.dma_start(out=xw[:], in_=xw16[b])
                gw = iop.tile([P, 32], I16, tag="gw")
                nc.sync.dma_start(out=gw[:], in_=gbw16[b])
                prt = iop.tile([P, SUB, 2], F32, tag="prt")
                nc.sync.dma_start(out=prt[:], in_=par[sl, :].rearrange("(s p) j -> p s j", p=P))
                esh = iop.tile([P, SUB, 3], F32, tag="esh")
                nc.sync.dma_start(out=esh[:], in_=eshift[sl, :].rearrange("(s p) j -> p s j", p=P))

                pg = geop.tile([P, 2 * SUB, 2 * PROW], F32, tag="pg")
                nc.gpsimd.dma_gather(
                    out_ap=pg[:], in_ap=pos_pair[:, :], idxs_ap=xw[:],
                    num_idxs=2 * BLK, num_idxs_reg=2 * BLK, elem_size=2 * PROW,
                )
                bcg = geop.tile([P, SUB, 64], F32, tag="bcg")
                nc.gpsimd.dma_gather(
                    out_ap=bcg[:], in_ap=cell64[:, :], idxs_ap=gw[:],
                    num_idxs=BLK, num_idxs_reg=BLK, elem_size=64,
                )

                # parity select: pos_i[p,s,0:4] = even/odd row half by parity
                pos1 = geop.tile([P, SUB, 4], F32, tag="pos1")
                pos2 = geop.tile([P, SUB, 4], F32, tag="pos2")
                posh = geop.tile([P, 2, SUB, 4], F32, tag="posh")
                pmsk = geop.tile([P, 2, SUB, 4], mybir.dt.uint8, tag="pmsk")
                nc.gpsimd.tensor_copy(out=pos1[:], in_=pg[:, 0:SUB, 0:4])
                nc.gpsimd.tensor_copy(out=pos2[:], in_=pg[:, SUB : 2 * SUB, 0:4])
                nc.gpsimd.tensor_copy(
                    out=posh[:].rearrange("p e s j -> p (e s) j"),
                    in_=pg[:, :, PROW : PROW + 4],
                )
                nc.gpsimd.tensor_copy(
                    out=pmsk[:],
                    in_=prt[:].transpose([0, 2, 1]).unsqueeze(3).to_broadcast([P, 2, SUB, 4]),
                )
                nc.vector.copy_predicated(
                    out=pos1[:].rearrange("p s j -> p (s j)"),
                    mask=pmsk[:, 0, :, :].rearrange("p s j -> p (s j)"),
                    data=posh[:, 0, :, :].rearrange("p s j -> p (s j)"),
                )
                nc.vector.copy_predicated(
                    out=pos2[:].rearrange("p s j -> p (s j)"),
                    mask=pmsk[:, 1, :, :].rearrange("p s j -> p (s j)"),
                    data=posh[:, 1, :, :].rearrange("p s j -> p (s j)"),
                )

                # tvec[p,s,j] = sum_i esh[p,s,i] * bc[p,s,3i+j]
                tvp = geop.tile([P, SUB, 3, 3], F32, tag="tvp")
                nc.vector.tensor_tensor(
                    out=tvp[:],
                    in0=esh[:].unsqueeze(3).to_broadcast([P, SUB, 3, 3]),
                    in1=bcg[:, :, 0:9].rearrange("p s (i j) -> p s i j", j=3),
                    op=OP.mult,
                )
                tv = geop.tile([P, SUB, 3], F32, tag="tv")
                nc.vector.reduce_sum(
                    out=tv[:], in_=tvp[:].transpose([0, 1, 3, 2]), axis=mybir.AxisListType.X,
                )
                rv = geop.tile([P, SUB, 3], F32, tag="rv")
                nc.vector.tensor_sub(rv[:], pos2[:, :, 0:3], pos1[:, :, 0:3])
                nc.vector.tensor_add(rv[:], rv[:], tv[:])
                rv2 = geop.tile([P, SUB, 3], F32, tag="rv2")
                nc.vector.tensor_mul(rv2[:], rv[:], rv[:])
                d2 = geop.tile([P, SUB], F32, tag="d2")
                nc.vector.reduce_sum(out=d2[:], in_=rv2[:], axis=mybir.AxisListType.X)
                nc.vector.tensor_scalar(
                    out=d2[:], in0=d2[:], scalar1=1e-12, scalar2=None, op0=OP.max,
                )
                ry = _newton_rsqrt(nc, geop, d2[:], SUB, magic_t, "rsq")
                dist = dist_store[:, b, :]
                nc.vector.tensor_mul(dist, d2[:], ry[:].bitcast(F32))

                rb = geop.tile([P, SUB, NB], F32, tag="rb")
                nc.vector.tensor_tensor(
                    out=rb[:],
                    in0=offs_sb[:].unsqueeze(1).to_broadcast([P, SUB, NB]),
                    in1=dist.unsqueeze(2).to_broadcast([P, SUB, NB]),
                    op=OP.subtract,
                )
                nc.scalar.activation(rb[:], rb[:], AF.Square, scale=sqc)
                nc.scalar.activation(rbf_store[:, b, :, :], rb[:], AF.Exp, scale=-1.0)

            if stage == "geo":
                for b in range(nblocks):
                    acc = accp.tile([P, SUB], F32, tag="acc")
                    nc.vector.tensor_copy(out=acc[:], in_=dist_store[:, b, :])
                    nc.sync.dma_start(
                        out=out[b * BLK : (b + 1) * BLK].rearrange("(s p) -> p s", p=P),
                        in_=acc[:],
                    )

            # =========== Phase B: gathers + TP + MLPs (silu table) ===========
            for b in range(nblocks if stage != "geo" else 0):
                e0 = b * BLK
                sl = slice(e0, e0 + BLK)
                xw = iop.tile([P, 64], I16, tag="xw")
                nc.sync.dma_start(out=xw[:], in_=xw16[b])
                prt = iop.tile([P, SUB, 2], F32, tag="prt")
                nc.sync.dma_start(out=prt[:], in_=par[sl, :].rearrange("(s p) j -> p s j", p=P))

                xg = gbigp.tile([P, 2 * SUB, 2 * XROW], F32, tag="xg")
                nc.gpsimd.dma_gather(
                    out_ap=xg[:], in_ap=nodes_pair[:, :], idxs_ap=xw[:],
                    num_idxs=2 * BLK, num_idxs_reg=2 * BLK, elem_size=2 * XROW,
                )
                x1 = gbigp.tile([P, SUB, 124], F32, tag="x1")
                x2 = gbigp.tile([P, SUB, 124], F32, tag="x2")
                xh = gbigp.tile([P, 2, SUB, 124], F32, tag="xh")
                xmsk = gbigp.tile([P, 2, SUB, 124], mybir.dt.uint8, tag="xmsk")
                nc.gpsimd.tensor_copy(out=x1[:], in_=xg[:, 0:SUB, 0:124])
                nc.gpsimd.tensor_copy(out=x2[:], in_=xg[:, SUB : 2 * SUB, 0:124])
                nc.gpsimd.tensor_copy(
                    out=xh[:].rearrange("p e s j -> p (e s) j"),
                    in_=xg[:, :, XROW : XROW + 124],
                )
                nc.gpsimd.tensor_copy(
                    out=xmsk[:],
                    in_=prt[:].transpose([0, 2, 1]).unsqueeze(3).to_broadcast([P, 2, SUB, 124]),
                )
                nc.vector.copy_predicated(
                    out=x1[:].rearrange("p s j -> p (s j)"),
                    mask=xmsk[:, 0, :, :].rearrange("p s j -> p (s j)"),
                    data=xh[:, 0, :, :].rearrange("p s j -> p (s j)"),
                )
                nc.vector.copy_predicated(
                    out=x2[:].rearrange("p s j -> p (s j)"),
                    mask=xmsk[:, 1, :, :].rearrange("p s j -> p (s j)"),
                    data=xh[:, 1, :, :].rearrange("p s j -> p (s j)"),
                )

                dist = dist_store[:, b, :]
                dc = geop.tile([P, SUB], F32, tag="dc")
                nc.vector.tensor_scalar(
                    out=dc[:], in0=dist, scalar1=CUTOFF, scalar2=None, op0=OP.min,
                )
                cosd = geop.tile([P, SUB], F32, tag="cosd")
                nc.scalar.activation(
                    cosd[:], dc[:], AF.Sin,
                    bias=nhalfpi_t[:, 0:1], scale=float(np.pi / CUTOFF),
                )
                mask = geop.tile([P, SUB], F32, tag="mask")
                nc.vector.tensor_scalar(
                    out=mask[:], in0=dist, scalar1=CUTOFF, scalar2=None, op0=OP.is_lt,
                )
                env = geop.tile([P, SUB], F32, tag="env")
                nc.vector.tensor_scalar(
                    out=env[:], in0=cosd[:], scalar1=-0.5, scalar2=0.5,
                    op0=OP.mult, op1=OP.add,
                )
                nc.vector.tensor_mul(env[:], env[:], mask[:])
                demb = geop.tile([P, SUB, NB], BF16, tag="demb")
                nc.vector.tensor_tensor(
                    out=demb[:], in0=rbf_store[:, b, :, :],
                    in1=env[:].unsqueeze(2).to_broadcast([P, SUB, NB]),
                    op=OP.mult,
                )

                if stage == "gather":
                    acc = accp.tile([P, SUB], F32, tag="acc")
                    nc.vector.reduce_sum(out=acc[:], in_=x1[:], axis=mybir.AxisListType.X)
                    nc.sync.dma_start(out=out[sl].rearrange("(s p) -> p s", p=P), in_=acc[:])
                    continue

                psmix = ps_mm.tile([P, SUB, NS], F32, tag="psmix")
                muv = geop.tile([P, SUB], F32, tag="muv")
                varv = geop.tile([P, SUB], F32, tag="varv")

                # ---- pass 1: tensor product per sub-tile ----
                for s in range(SUB):
                    ptb = pfp.tile([P, KPAD], BF16, tag="ptb")
                    nc.vector.memset(ptb[:, KTP:KPAD], 0.0)
                    a1 = x1[:, s, 0:L0]
                    a2 = x2[:, s, 0:L0]
                    nc.vector.tensor_tensor(
                        out=ptb[:, 0:1024].rearrange("p (u v) -> p u v", v=L0),
                        in0=a1.unsqueeze(2).to_broadcast([P, L0, L0]),
                        in1=a2.unsqueeze(1).to_broadcast([P, L0, L0]),
                        op=OP.mult,
                    )
                    b1 = x1[:, s, 32:80].rearrange("p (u m) -> p u m", m=3)
                    b2 = x2[:, s, 32:80].rearrange("p (u m) -> p u m", m=3)
                    pb = workp.tile([P, L1, L1, 3], F32, tag="pb")
                    nc.vector.tensor_tensor(
                        out=pb[:],
                        in0=b1.unsqueeze(2).to_broadcast([P, L1, L1, 3]),
                        in1=b2.unsqueeze(1).to_broadcast([P, L1, L1, 3]),
                        op=OP.mult,
                    )
                    pf = workp.tile([P, 320], F32, tag="pf")
                    nc.vector.reduce_sum(
                        out=pf[:, 0:256].rearrange("p (u v) -> p u v", v=L1),
                        in_=pb[:], axis=mybir.AxisListType.X,
                    )
                    c1 = x1[:, s, 80:120].rearrange("p (u m) -> p u m", m=5)
                    c2 = x2[:, s, 80:120].rearrange("p (u m) -> p u m", m=5)
                    pc = workp.tile([P, L2, L2, 5], F32, tag="pc")
                    nc.vector.tensor_tensor(
                        out=pc[:],
                        in0=c1.unsqueeze(2).to_broadcast([P, L2, L2, 5]),
                        in1=c2.unsqueeze(1).to_broadcast([P, L2, L2, 5]),
                        op=OP.mult,
                    )
                    nc.vector.reduce_sum(
                        out=pf[:, 256:320].rearrange("p (u v) -> p u v", v=L2),
                        in_=pc[:], axis=mybir.AxisListType.X,
                    )
                    nc.vector.tensor_copy(out=ptb[:, 1024:1344], in_=pf[:])

                    # transposes in groups of <=4 chunks -> one PSUM bank,
                    # one batched PSUM->SBUF copy per group
                    for g, chunks in enumerate(((0, 1, 2, 3), (4, 5, 6, 7), (8, 9, 10))):
                        ptp = ps_tr.tile([P, 4, P], BF16, tag="ptp")
                        for j, c in enumerate(chunks):
                            nc.tensor.transpose(
                                ptp[:, j, :], ptb[:, c * P : (c + 1) * P], identb[:]
                            )
                        pts = trsbp.tile([P, 4, P], BF16, tag="pts")
                        ncopy = len(chunks)
                        if g == 1:
                            nc.scalar.copy(pts[:, 0:ncopy, :], ptp[:, 0:ncopy, :])
                        else:
                            nc.vector.tensor_copy(pts[:, 0:ncopy, :], ptp[:, 0:ncopy, :])
                        for j, c in enumerate(chunks):
                            nc.tensor.matmul(
                                psmix[:, s, :], lhsT=pts[:, j, :], rhs=w_sb[:, c, :],
                                start=(c == 0), stop=(c == NCHUNK - 1),
                            )

                    stats = geop.tile([P, 6], F32, tag="stats")
                    nc.vector.bn_stats(out=stats[:], in_=psmix[:, s, :])
                    mv = geop.tile([P, 2], F32, tag="mv")
                    nc.vector.bn_aggr(out=mv[:], in_=stats[:])
                    nc.vector.tensor_copy(out=muv[:, s : s + 1], in_=mv[:, 0:1])
                    nc.vector.tensor_copy(out=varv[:, s : s + 1], in_=mv[:, 1:2])

                if stage == "tp":
                    acc = accp.tile([P, SUB], F32, tag="acc")
                    nc.vector.tensor_copy(out=acc[:], in_=muv[:])
                    nc.sync.dma_start(out=out[sl].rearrange("(s p) -> p s", p=P), in_=acc[:])
                    continue

                # ---- block-level LN rstd ----
                nc.vector.tensor_scalar(
                    out=varv[:], in0=varv[:], scalar1=1e-5, scalar2=None, op0=OP.add,
                )
                ryl = _newton_rsqrt(nc, geop, varv[:], SUB, magic_t, "lnr")
                rstd = ryl[:].bitcast(F32)
                tb = geop.tile([P, SUB], F32, tag="tb")
                nc.vector.tensor_mul(tb[:], muv[:], rstd)
                nc.vector.tensor_scalar(
                    out=tb[:], in0=tb[:], scalar1=-1.0, scalar2=None, op0=OP.mult,
                )

                acc = accp.tile([P, SUB], F32, tag="acc")

                # ---- pass 2: LN apply + dfilter + final MLP ----
                for s in range(SUB):
                    ynorm = workp.tile([P, NS], BF16, tag="ynorm")
                    nc.scalar.activation(
                        ynorm[:], psmix[:, s, :], AF.Identity,
                        bias=tb[:, s : s + 1], scale=rstd[:, s : s + 1],
                    )

                    dT_ps = ps_tr.tile([P, 4, P], BF16, tag="ptp")
                    nc.tensor.transpose(dT_ps[0:NB, 0, :], demb[:, s, :], identb[:])
                    dT = trsbp.tile([NB, P], BF16, tag="dT")
                    nc.scalar.copy(dT[:], dT_ps[0:NB, 0, :])
                    ph = ps_h.tile([P, 128], F32, tag="ph")
                    nc.tensor.matmul(ph[:], lhsT=dT[:], rhs=dfw1_sb[:], start=True, stop=True)
                    nc.vector.tensor_add(ph[:], ph[:], dfb1_rep[:])
                    sact = workp.tile([P, 128], BF16, tag="sact")
                    nc.scalar.activation(sact[:], ph[:], AF.Silu)
                    sT_ps = ps_tr.tile([P, 4, P], BF16, tag="ptp")
                    nc.tensor.transpose(sT_ps[:, 0, :], sact[:], identb[:])
                    sT = trsbp.tile([P, P], BF16, tag="sT")
                    nc.vector.tensor_copy(sT[:], sT_ps[:, 0, :])
                    pdf = ps_df.tile([P, 256], F32, tag="pdf")
                    nc.tensor.matmul(pdf[:], lhsT=sT[:], rhs=dfw2gb_sb[:], start=True, stop=True)
                    dfs = workp.tile([P, 256], BF16, tag="dfs")
                    nc.vector.tensor_add(dfs[:], pdf[:], dfb2gb_rep[:])

                    rg = workp.tile([P, 128], BF16, tag="rg")
                    nc.vector.tensor_mul(rg[:], ynorm[:], dfs[:, 0:128])
                    nc.vector.tensor_add(rg[:], rg[:], dfs[:, 128:256])

                    rT_ps = ps_tr.tile([P, 4, P], BF16, tag="ptp")
                    nc.tensor.transpose(rT_ps[:, 0, :], rg[:], identb[:])
                    rT = trsbp.tile([P, P], BF16, tag="rT")
                    nc.scalar.copy(rT[:], rT_ps[:, 0, :])
                    pg2 = ps_g.tile([P, 512], F32, tag="pg")
                    nc.tensor.matmul(pg2[:], lhsT=rT[:], rhs=mlpw1_sb[:], start=True, stop=True)
                    nc.vector.tensor_add(pg2[:], pg2[:], mlpb1_rep[:])
                    gact = gbigp.tile([P, 512], BF16, tag="gact")
                    nc.scalar.activation(gact[:], pg2[:], AF.Silu)
                    scr = gbigp.tile([P, 512], BF16, tag="scr")
                    nc.vector.tensor_mul(scr[:], gact[:], w2rep_sb[:])
                    nc.vector.reduce_sum(
                        out=acc[:, s : s + 1], in_=scr[:], axis=mybir.AxisListType.X,
                    )

                nc.vector.tensor_scalar(
                    out=acc[:], in0=acc[:], scalar1=b2_sb[:, 0:1], scalar2=None,
                    op0=OP.add,
                )
                nc.sync.dma_start(out=out[sl].rearrange("(s p) -> p s", p=P), in_=acc[:])

    nc.compile()
    return nc


def _get_compiled():
    global _compiled
    if _compiled is None:
        _compiled = _build(NBLOCKS)
    return _compiled


def _wrap16(idx_block):
    """int array [512] -> dma_gather wrapped int16 layout [128, 32]
    (index j at [j%16, j//16], replicated across the 8 gpsimd cores)."""
    w = idx_block.astype(np.int16).reshape(-1, 16).T  # [16, n/16]
    return np.tile(w, (8, 1))


def _prep(inputs):
    nodes = np.asarray(inputs["nodes"], np.float32)
    edge_index = np.asarray(inputs["edge_index"]).astype(np.int64)
    graph_batch = np.asarray(inputs["graph_batch"]).astype(np.int64)
    cell = np.asarray(inputs["cell"], np.float32)
    edge_shift = np.asarray(inputs["edge_shift"], np.float32)
    pos = np.asarray(inputs["pos"], np.float32)

    nodes_pad = np.zeros((N_NODES, XROW), np.float32)
    nodes_pad[:, :NODE_DIM] = nodes
    nodes_pad[:, 120:123] = pos
    nodes_pad[:, 123] = graph_batch
    nodes_pair = nodes_pad.reshape(NPAIR, 2 * XROW)

    pos_pad = np.zeros((N_NODES, PROW), np.float32)
    pos_pad[:, 0:3] = pos
    pos_pair = pos_pad.reshape(NPAIR, 2 * PROW)

    cell64 = np.zeros((32, 64), np.float32)
    cell64[:, 0:9] = cell.reshape(32, 9)

    alpha = 1.0 / np.sqrt(float(L0 * L0 + L1 * L1 + L2 * L2))
    w0 = np.asarray(inputs["W0"], np.float32).reshape(L0 * L0, NS) * alpha
    w1 = np.asarray(inputs["W1"], np.float32).reshape(L1 * L1, NS) * (alpha / np.sqrt(3.0))
    w2 = np.asarray(inputs["W2"], np.float32).reshape(L2 * L2, NS) * (alpha / np.sqrt(5.0))
    wflat = np.zeros((KPAD, NS), np.float32)
    wflat[0:1024] = w0
    wflat[1024:1280] = w1
    wflat[1280:1344] = w2

    ln_g = np.asarray(inputs["ln_g"], np.float32)
    ln_b = np.asarray(inputs["ln_b"], np.float32)
    df_w2 = np.asarray(inputs["df_w2"], np.float32)
    df_b2 = np.asarray(inputs["df_b2"], np.float32)
    dfw2gb = np.concatenate([df_w2 * ln_g[None, :], df_w2 * ln_b[None, :]], axis=1)
    dfb2gb = np.concatenate([df_b2 * ln_g, df_b2 * ln_b])[None, :]

    bf = lambda a: np.ascontiguousarray(a).astype(ml_dtypes.bfloat16)

    common = {
        "nodes_pair": nodes_pair,
        "pos_pair": pos_pair,
        "cell64": cell64,
        "wflat": bf(wflat),
        "dfw1": bf(np.asarray(inputs["df_w1"], np.float32)),
        "dfb1": np.asarray(inputs["df_b1"], np.float32)[None, :],
        "dfw2gb": bf(dfw2gb),
        "dfb2gb": np.ascontiguousarray(dfb2gb.astype(np.float32)),
        "mlpw1": bf(np.asarray(inputs["mlp_w1"], np.float32)),
        "mlpb1": np.asarray(inputs["mlp_b1"], np.float32)[None, :],
        "w2row": bf(np.asarray(inputs["mlp_w2"], np.float32).T),
        "b2sc": np.asarray(inputs["mlp_b2"], np.float32).reshape(1, 1),
        "offs": np.linspace(0.0, CUTOFF, NB, dtype=np.float32)[None, :],
    }

    nblocks = E_PAD // BLK
    in_maps = []
    for c in range(NCORES):
        lo, hi = c * E_CORE, (c + 1) * E_CORE
        src = np.zeros(E_PAD, np.int64)
        dst = np.zeros(E_PAD, np.int64)
        esh = np.zeros((E_PAD, 3), np.float32)
        src[: hi - lo] = edge_index[0, lo:hi]
        dst[: hi - lo] = edge_index[1, lo:hi]
        esh[: hi - lo] = edge_shift[lo:hi]

        xw = np.zeros((nblocks, P, 64), np.int16)
        gbw = np.zeros((nblocks, P, 32), np.int16)
        for b in range(nblocks):
            sb = src[b * BLK : (b + 1) * BLK]
            db = dst[b * BLK : (b + 1) * BLK]
            xw[b, :, 0:32] = _wrap16(sb >> 1)
            xw[b, :, 32:64] = _wrap16(db >> 1)
            gbw[b] = _wrap16(graph_batch[sb])
        parr = np.stack([(src & 1), (dst & 1)], axis=1).astype(np.float32)

        m = dict(common)
        m["xw16"] = xw
        m["gbw16"] = gbw
        m["par"] = parr
        m["eshift"] = esh
        in_maps.append(m)
    return in_maps


def kernel(**inputs) -> np.ndarray:
    nc = _get_compiled()
    in_maps = _prep(inputs)
    res = run_bass_kernel_spmd(nc, in_maps, core_ids=list(range(NCORES)))
    outs = [res.results[c]["out"][:E_CORE] for c in range(NCORES)]
    return np.concatenate(outs).reshape(N_EDGES, 1).astype(np.float32)

